# revision 4
# baseline (speedup 1.0000x reference)
import sys

if "/opt/trn_rl_repo" not in sys.path:
    sys.path.insert(0, "/opt/trn_rl_repo")

import numpy as np

B, S, D, H = 8, 1024, 512, 8
DK = 64
NEG = np.float32(-1e9)

_CACHE = {}


def _build():
    import concourse.bacc as bacc
    import concourse.mybir as mybir
    import concourse.tile as tile
    from concourse.masks import make_identity

    f16 = mybir.dt.float16
    f32 = mybir.dt.float32
    f32r = mybir.dt.float32r
    AF = mybir.ActivationFunctionType
    OP = mybir.AluOpType

    nc = bacc.Bacc("TRN2", target_bir_lowering=False, debug=False)

    qb_d = nc.dram_tensor("qb", (S, D), f16, kind="ExternalInput")
    kb_d = nc.dram_tensor("kb", (S, D), f16, kind="ExternalInput")
    vb_d = nc.dram_tensor("vb", (S, D), f16, kind="ExternalInput")
    qres_d = nc.dram_tensor("qres", (S, D), f32, kind="ExternalInput")
    madj_d = nc.dram_tensor("madj", (S, S), f32r, kind="ExternalInput")
    madjT_d = nc.dram_tensor("madjT", (S, S), f32r, kind="ExternalInput")
    wq_d = nc.dram_tensor("wq", (D, D), f16, kind="ExternalInput")
    wk_d = nc.dram_tensor("wk", (D, D), f16, kind="ExternalInput")
    wv_d = nc.dram_tensor("wv", (D, D), f16, kind="ExternalInput")
    wo_d = nc.dram_tensor("wo", (D, D), f16, kind="ExternalInput")
    bq_d = nc.dram_tensor("bqc", (128, 4), f32, kind="ExternalInput")
    bk_d = nc.dram_tensor("bkc", (128, 4), f32, kind="ExternalInput")
    bv_d = nc.dram_tensor("bvr", (1, D), f32, kind="ExternalInput")
    g_d = nc.dram_tensor("gr", (1, D), f32, kind="ExternalInput")
    lnb_d = nc.dram_tensor("lnbr", (1, D), f32, kind="ExternalInput")
    eye_d = nc.dram_tensor("eye", (128, 128), f32r, kind="ExternalInput")

    out_d = nc.dram_tensor("out_o", (S, D), f32, kind="ExternalOutput")
    attn_d = nc.dram_tensor("attn_o", (H, S, S), f32, kind="ExternalOutput")

    with tile.TileContext(nc) as tc:
        with tc.tile_pool(name="persist", bufs=1) as persist:
            qT = persist.tile([128, 4, S], f16)
            kT = persist.tile([128, 4, S], f16)
            v_sb = persist.tile([128, 8, D], f16)
            madj_sb = persist.tile([128, 8, S], f32r)
            madjT_sb = persist.tile([128, 8, S], f32r)
            qres_sb = persist.tile([128, 8, D], f32)
            ctxT_sb = persist.tile([128, 4, S], f16)
            wo_sb = persist.tile([128, 4, D], f16)
            gb_sb = persist.tile([128, D], f32)
            lnbb_sb = persist.tile([128, D], f32)
            bvb_sb = persist.tile([128, D], f32)
            eye_sb = persist.tile([128, 128], f32r)
            ident_sb = persist.tile([128, 128], f32)
            bq_sb = persist.tile([128, 4], f32)
            bk_sb = persist.tile([128, 4], f32)
            eps_sb = persist.tile([128, 1], f32)
            nc.vector.memset(eps_sb[:], 1e-6)

            nc.sync.dma_start(
                out=madj_sb, in_=madj_d[:].rearrange("(qc p) k -> p qc k", p=128)
            )
            nc.sync.dma_start(
                out=madjT_sb, in_=madjT_d[:].rearrange("(kc p) q -> p kc q", p=128)
            )
            nc.sync.dma_start(
                out=qres_sb, in_=qres_d[:].rearrange("(sc p) d -> p sc d", p=128)
            )
            nc.sync.dma_start(
                out=wo_sb, in_=wo_d[:].rearrange("(c p) n -> p c n", p=128)
            )
            nc.gpsimd.dma_start(out=gb_sb, in_=g_d[:].to_broadcast([128, D]))
            nc.gpsimd.dma_start(out=lnbb_sb, in_=lnb_d[:].to_broadcast([128, D]))
            nc.gpsimd.dma_start(out=bvb_sb, in_=bv_d[:].to_broadcast([128, D]))
            nc.sync.dma_start(out=eye_sb, in_=eye_d[:])
            make_identity(nc, ident_sb[:])
            nc.sync.dma_start(out=bq_sb, in_=bq_d[:])
            nc.sync.dma_start(out=bk_sb, in_=bk_d[:])

            # ---------- projections ----------
            with tc.tile_pool(name="pin", bufs=1) as pin, tc.tile_pool(
                name="psp", bufs=2, space="PSUM"
            ) as psp:
                QT = pin.tile([128, 4, S], f16)
                KT = pin.tile([128, 4, S], f16)
                VT = pin.tile([128, 4, S], f16)
                wq_sb = pin.tile([128, 4, D], f16)
                wk_sb = pin.tile([128, 4, D], f16)
                wv_sb = pin.tile([128, 4, D], f16)
                nc.sync.dma_start(
                    out=wq_sb, in_=wq_d[:].rearrange("(c p) n -> p c n", p=128)
                )
                nc.sync.dma_start(
                    out=wk_sb, in_=wk_d[:].rearrange("(c p) n -> p c n", p=128)
                )
                nc.sync.dma_start(
                    out=wv_sb, in_=wv_d[:].rearrange("(c p) n -> p c n", p=128)
                )
                for dc in range(4):
                    nc.sync.dma_start_transpose(
                        QT[:, dc], qb_d[:, dc * 128 : (dc + 1) * 128]
                    )
                    nc.sync.dma_start_transpose(
                        KT[:, dc], kb_d[:, dc * 128 : (dc + 1) * 128]
                    )
                    nc.sync.dma_start_transpose(
                        VT[:, dc], vb_d[:, dc * 128 : (dc + 1) * 128]
                    )

                for src, wsb, bsb, dst in (
                    (QT, wq_sb, bq_sb, qT),
                    (KT, wk_sb, bk_sb, kT),
                ):
                    for hc in range(4):
                        for sh in range(2):
                            ps = psp.tile([128, 512], f32, tag="proj")
                            for dc in range(4):
                                nc.tensor.matmul(
                                    ps,
                                    wsb[:, dc, hc * 128 : (hc + 1) * 128],
                                    src[:, dc, sh * 512 : (sh + 1) * 512],
                                    start=(dc == 0),
                                    stop=(dc == 3),
                                )
                            nc.vector.tensor_scalar(
                                out=dst[:, hc, sh * 512 : (sh + 1) * 512],
                                in0=ps,
                                scalar1=bsb[:, hc : hc + 1],
                                scalar2=None,
                                op0=OP.add,
                            )
                for sc in range(8):
                    ps = psp.tile([128, 512], f32, tag="proj")
                    for dc in range(4):
                        nc.tensor.matmul(
                            ps,
                            VT[:, dc, sc * 128 : (sc + 1) * 128],
                            wv_sb[:, dc],
                            start=(dc == 0),
                            stop=(dc == 3),
                        )
                    nc.vector.tensor_tensor(
                        out=v_sb[:, sc], in0=ps, in1=bvb_sb, op=OP.add
                    )

            # ---------- attention (per head) ----------
            with tc.tile_pool(name="hp", bufs=2) as hp, tc.tile_pool(
                name="sps", bufs=2, space="PSUM"
            ) as sps, tc.tile_pool(name="cps", bufs=2, space="PSUM") as cps, tc.tile_pool(
                name="aps", bufs=1, space="PSUM"
            ) as aps, tc.tile_pool(name="dstage", bufs=2, space="DRAM") as dstage:
                for h in range(H):
                    hc, hr = h // 2, (h % 2) * 64
                    rs = hp.tile([128, 8], f32, tag="rs")
                    # natural-orientation scores -> attn output
                    for qc in range(8):
                        ps = sps.tile([128, 1024], f32, tag="sc")
                        for kh in range(2):
                            nc.tensor.matmul(
                                ps[:, kh * 512 : (kh + 1) * 512],
                                qT[hr : hr + 64, hc, qc * 128 : (qc + 1) * 128],
                                kT[hr : hr + 64, hc, kh * 512 : (kh + 1) * 512],
                                start=True,
                                stop=False,
                            )
                            nc.tensor.matmul(
                                ps[:, kh * 512 : (kh + 1) * 512],
                                eye_sb,
                                madj_sb[:, qc, kh * 512 : (kh + 1) * 512],
                                start=False,
                                stop=True,
                            )
                        E = hp.tile([128, S], f16, tag="E")
                        nc.scalar.activation(
                            out=E,
                            in_=ps,
                            func=AF.Exp,
                            accum_out=rs[:, qc : qc + 1],
                        )
                        rcp = hp.tile([128, 1], f32, tag="rcp")
                        nc.vector.reciprocal(out=rcp, in_=rs[:, qc : qc + 1])
                        ast = hp.tile([128, S], f32, tag="ast")
                        nc.vector.tensor_scalar(
                            out=ast,
                            in0=E,
                            scalar1=rcp,
                            scalar2=None,
                            op0=OP.mult,
                        )
                        nc.sync.dma_start(
                            out=attn_d[h, qc * 128 : (qc + 1) * 128, :], in_=ast
                        )

                    # per-query reciprocal row, broadcast to 64 partitions
                    psT = aps.tile([8, 128], f32, tag="rsT")
                    nc.tensor.transpose(psT, rs, ident_sb)
                    rc = hp.tile([8, 128], f32, tag="rc")
                    nc.vector.reciprocal(out=rc, in_=psT)
                    rrow = dstage.tile([1, 1024], f32, tag="rrow")
                    nc.sync.dma_start(
                        out=rrow[:].rearrange("o (a b) -> (o a) b", a=8), in_=rc
                    )
                    rb = hp.tile([64, 1024], f32, tag="rb")
                    nc.gpsimd.dma_start(out=rb, in_=rrow[:].to_broadcast([64, 1024]))

                    # transposed-orientation scores -> exp -> ET
                    ET = hp.tile([128, 8, S], f16, tag="ET")
                    for kc in range(8):
                        ps = sps.tile([128, 1024], f32, tag="sc")
                        for qh in range(2):
                            nc.tensor.matmul(
                                ps[:, qh * 512 : (qh + 1) * 512],
                                kT[hr : hr + 64, hc, kc * 128 : (kc + 1) * 128],
                                qT[hr : hr + 64, hc, qh * 512 : (qh + 1) * 512],
                                start=True,
                                stop=False,
                            )
                            nc.tensor.matmul(
                                ps[:, qh * 512 : (qh + 1) * 512],
                                eye_sb,
                                madjT_sb[:, kc, qh * 512 : (qh + 1) * 512],
                                start=False,
                                stop=True,
                            )
                        nc.scalar.activation(out=ET[:, kc], in_=ps, func=AF.Exp)

                    # ctx^T = v^T @ E^T, normalized by 1/rowsum
                    for qh in range(2):
                        pc = cps.tile([64, 512], f32, tag="ctx")
                        for kc in range(8):
                            nc.tensor.matmul(
                                pc,
                                v_sb[:, kc, h * 64 : (h + 1) * 64],
                                ET[:, kc, qh * 512 : (qh + 1) * 512],
                                start=(kc == 0),
                                stop=(kc == 7),
                            )
                        nc.vector.tensor_tensor(
                            out=ctxT_sb[hr : hr + 64, hc, qh * 512 : (qh + 1) * 512],
                            in0=pc,
                            in1=rb[:, qh * 512 : (qh + 1) * 512],
                            op=OP.mult,
                        )

            # ---------- output projection + residual + LayerNorm ----------
            with tc.tile_pool(name="op", bufs=2) as op, tc.tile_pool(
                name="ops", bufs=2, space="PSUM"
            ) as ops:
                for sc in range(8):
                    po = ops.tile([128, 512], f32, tag="o")
                    for c in range(4):
                        nc.tensor.matmul(
                            po,
                            ctxT_sb[:, c, sc * 128 : (sc + 1) * 128],
                            wo_sb[:, c],
                            start=(c == 0),
                            stop=(c == 3),
                        )
                    x = op.tile([128, 512], f32, tag="x")
                    nc.vector.tensor_tensor(
                        out=x, in0=po, in1=qres_sb[:, sc], op=OP.add
                    )
                    st = op.tile([128, 6], f32, tag="st")
                    nc.vector.bn_stats(out=st, in_=x)
                    ag = op.tile([128, 2], f32, tag="ag")
                    nc.vector.bn_aggr(out=ag, in_=st)
                    sd = op.tile([128, 1], f32, tag="sd")
                    nc.scalar.activation(
                        out=sd, in_=ag[:, 1:2], func=AF.Sqrt, bias=eps_sb[:]
                    )
                    rstd = op.tile([128, 1], f32, tag="rstd")
                    nc.vector.reciprocal(out=rstd, in_=sd)
                    y = op.tile([128, 512], f32, tag="y")
                    nc.vector.tensor_scalar(
                        out=y,
                        in0=x,
                        scalar1=ag[:, 0:1],
                        scalar2=rstd,
                        op0=OP.subtract,
                        op1=OP.mult,
                    )
                    y2 = op.tile([128, 512], f32, tag="y2")
                    nc.vector.tensor_tensor(out=y2, in0=y, in1=gb_sb, op=OP.mult)
                    o = op.tile([128, 512], f32, tag="oo")
                    nc.vector.tensor_tensor(out=o, in0=y2, in1=lnbb_sb, op=OP.add)
                    nc.sync.dma_start(out=out_d[sc * 128 : (sc + 1) * 128, :], in_=o)

    nc.finalize()
    return nc


def _get_program():
    if "nc" not in _CACHE:
        _CACHE["nc"] = _build()
    return _CACHE["nc"]


def make_in_maps(Q, K, V, attn_mask, adjoin_matrix, Wq, bq, Wk, bk, Wv, bv, Wo, bo, ln_g, ln_b):
    scale = np.float32(0.125)
    wq_s = (np.asarray(Wq, np.float32) * scale).astype(np.float16)
    wk_s = np.asarray(Wk, np.float32).astype(np.float16)
    wv_s = np.asarray(Wv, np.float32).astype(np.float16)
    wo_s = np.asarray(Wo, np.float32).astype(np.float16)
    bqc = np.ascontiguousarray(
        (np.asarray(bq, np.float32) * scale).reshape(4, 128).T
    )
    bkc = np.ascontiguousarray(np.asarray(bk, np.float32).reshape(4, 128).T)
    bvr = np.asarray(bv, np.float32).reshape(1, D)
    gr = np.asarray(ln_g, np.float32).reshape(1, D)
    lnbr = np.asarray(ln_b, np.float32).reshape(1, D)
    eye = np.eye(128, dtype=np.float32)
    bo32 = np.asarray(bo, np.float32)

    madj_all = np.where(np.asarray(attn_mask, bool), NEG, np.float32(0.0)).astype(
        np.float32
    ) + np.asarray(adjoin_matrix, np.float32)[:, 0]

    in_maps = []
    for b in range(B):
        in_maps.append(
            {
                "qb": np.asarray(Q[b], np.float32).astype(np.float16),
                "kb": np.asarray(K[b], np.float32).astype(np.float16),
                "vb": np.asarray(V[b], np.float32).astype(np.float16),
                "qres": (np.asarray(Q[b], np.float32) + bo32[None, :]).astype(
                    np.float32
                ),
                "madj": np.ascontiguousarray(madj_all[b]),
                "madjT": np.ascontiguousarray(madj_all[b].T),
                "wq": wq_s,
                "wk": wk_s,
                "wv": wv_s,
                "wo": wo_s,
                "bqc": bqc,
                "bkc": bkc,
                "bvr": bvr,
                "gr": gr,
                "lnbr": lnbr,
                "eye": eye,
            }
        )
    return in_maps


def kernel(Q, K, V, attn_mask, adjoin_matrix, Wq, bq, Wk, bk, Wv, bv, Wo, bo, ln_g, ln_b):
    from concourse import bass_utils

    nc = _get_program()
    in_maps = make_in_maps(
        Q, K, V, attn_mask, adjoin_matrix, Wq, bq, Wk, bk, Wv, bv, Wo, bo, ln_g, ln_b
    )
    res = bass_utils.run_bass_kernel_spmd(nc, in_maps, core_ids=list(range(B)))
    out = np.stack([r["out_o"] for r in res.results])
    attn = np.stack([r["attn_o"] for r in res.results])
    return out, attn


# revision 23
# speedup vs baseline: 1.0226x; 1.0226x over previous
import sys

if "/opt/trn_rl_repo" not in sys.path:
    sys.path.insert(0, "/opt/trn_rl_repo")

import numpy as np

B, S, D, H = 8, 1024, 512, 8
DK = 64
NEG = np.float32(-1e9)

_CACHE = {}


def _build():
    import concourse.bacc as bacc
    import concourse.mybir as mybir
    import concourse.tile as tile
    from concourse.masks import make_identity

    f16 = mybir.dt.float16
    f32 = mybir.dt.float32
    f32r = mybir.dt.float32r
    AF = mybir.ActivationFunctionType
    OP = mybir.AluOpType

    nc = bacc.Bacc("TRN2", target_bir_lowering=False, debug=False)

    qb_d = nc.dram_tensor("qb", (S, D), f16, kind="ExternalInput")
    kb_d = nc.dram_tensor("kb", (S, D), f16, kind="ExternalInput")
    vb_d = nc.dram_tensor("vb", (S, D), f16, kind="ExternalInput")
    qres_d = nc.dram_tensor("qres", (S, D), f32, kind="ExternalInput")
    emadj_d = nc.dram_tensor("emadj", (S, S), f16, kind="ExternalInput")
    emadjT_d = nc.dram_tensor("emadjT", (S, S), f16, kind="ExternalInput")
    wq_d = nc.dram_tensor("wq", (D, D), f16, kind="ExternalInput")
    wk_d = nc.dram_tensor("wk", (D, D), f16, kind="ExternalInput")
    wv_d = nc.dram_tensor("wv", (D, D), f16, kind="ExternalInput")
    wo_d = nc.dram_tensor("wo", (D, D), f16, kind="ExternalInput")
    bq_d = nc.dram_tensor("bqc", (128, 4), f32, kind="ExternalInput")
    bk_d = nc.dram_tensor("bkc", (128, 4), f32, kind="ExternalInput")
    bv_d = nc.dram_tensor("bvr", (1, D), f32, kind="ExternalInput")
    g_d = nc.dram_tensor("gr", (1, D), f32, kind="ExternalInput")
    lnb_d = nc.dram_tensor("lnbr", (1, D), f32, kind="ExternalInput")

    out_d = nc.dram_tensor("out_o", (S, D), f32, kind="ExternalOutput")
    attn_d = nc.dram_tensor("attn_o", (H, S, S), f32, kind="ExternalOutput")

    with tile.TileContext(nc) as tc:
        with tc.tile_pool(name="persist", bufs=1) as persist:
            qT = persist.tile([128, 4, S], f16)
            kT = persist.tile([128, 4, S], f16)
            v65 = persist.tile([128, 8, 8, 65], f16)
            emadj_sb = persist.tile([128, 8, S], f16)
            emadjT_sb = persist.tile([128, 8, S], f16)
            qres_sb = persist.tile([128, 8, D], f32)
            ctxT_sb = persist.tile([128, 4, S], f16)
            wo_sb = persist.tile([128, 4, D], f16)
            gb_sb = persist.tile([128, D], f32)
            lnbb_sb = persist.tile([128, D], f32)
            bvb_sb = persist.tile([128, D], f32)
            ident_sb = persist.tile([128, 128], f32)
            bq_sb = persist.tile([128, 4], f32)
            bk_sb = persist.tile([128, 4], f32)
            eps_sb = persist.tile([128, 1], f32)
            nc.vector.memset(eps_sb[:], 1e-6)
            nc.vector.memset(v65[:], 1.0)

            nc.sync.dma_start(
                out=emadj_sb, in_=emadj_d[:].rearrange("(qc p) k -> p qc k", p=128)
            )
            nc.sync.dma_start(
                out=emadjT_sb, in_=emadjT_d[:].rearrange("(kc p) q -> p kc q", p=128)
            )
            nc.sync.dma_start(
                out=qres_sb, in_=qres_d[:].rearrange("(sc p) d -> p sc d", p=128)
            )
            nc.sync.dma_start(
                out=wo_sb, in_=wo_d[:].rearrange("(c p) n -> p c n", p=128)
            )
            nc.gpsimd.dma_start(out=gb_sb, in_=g_d[:].to_broadcast([128, D]))
            nc.gpsimd.dma_start(out=lnbb_sb, in_=lnb_d[:].to_broadcast([128, D]))
            nc.gpsimd.dma_start(out=bvb_sb, in_=bv_d[:].to_broadcast([128, D]))
            make_identity(nc, ident_sb[:])
            nc.sync.dma_start(out=bq_sb, in_=bq_d[:])
            nc.sync.dma_start(out=bk_sb, in_=bk_d[:])

            # ---------- projections ----------
            with tc.tile_pool(name="pin", bufs=1) as pin, tc.tile_pool(
                name="psp", bufs=2, space="PSUM"
            ) as psp:
                QT = pin.tile([128, 4, S], f16)
                KT = pin.tile([128, 4, S], f16)
                VT = pin.tile([128, 4, S], f16)
                wq_sb = pin.tile([128, 4, D], f16)
                wk_sb = pin.tile([128, 4, D], f16)
                wv_sb = pin.tile([128, 4, D], f16)
                nc.sync.dma_start(
                    out=wq_sb, in_=wq_d[:].rearrange("(c p) n -> p c n", p=128)
                )
                nc.sync.dma_start(
                    out=wk_sb, in_=wk_d[:].rearrange("(c p) n -> p c n", p=128)
                )
                nc.sync.dma_start(
                    out=wv_sb, in_=wv_d[:].rearrange("(c p) n -> p c n", p=128)
                )
                for dc in range(4):
                    nc.sync.dma_start_transpose(
                        QT[:, dc], qb_d[:, dc * 128 : (dc + 1) * 128]
                    )
                    nc.sync.dma_start_transpose(
                        KT[:, dc], kb_d[:, dc * 128 : (dc + 1) * 128]
                    )
                    nc.sync.dma_start_transpose(
                        VT[:, dc], vb_d[:, dc * 128 : (dc + 1) * 128]
                    )

                for src, wsb, bsb, dst in (
                    (QT, wq_sb, bq_sb, qT),
                    (KT, wk_sb, bk_sb, kT),
                ):
                    for hc in range(4):
                        for sh in range(2):
                            ps = psp.tile([128, 512], f32, tag="proj")
                            for dc in range(4):
                                nc.tensor.matmul(
                                    ps,
                                    wsb[:, dc, hc * 128 : (hc + 1) * 128],
                                    src[:, dc, sh * 512 : (sh + 1) * 512],
                                    start=(dc == 0),
                                    stop=(dc == 3),
                                )
                            nc.vector.tensor_scalar(
                                out=dst[:, hc, sh * 512 : (sh + 1) * 512],
                                in0=ps,
                                scalar1=bsb[:, hc : hc + 1],
                                scalar2=None,
                                op0=OP.add,
                            )
                for sc in range(8):
                    ps = psp.tile([128, 512], f32, tag="proj")
                    for dc in range(4):
                        nc.tensor.matmul(
                            ps,
                            VT[:, dc, sc * 128 : (sc + 1) * 128],
                            wv_sb[:, dc],
                            start=(dc == 0),
                            stop=(dc == 3),
                        )
                    nc.vector.tensor_tensor(
                        out=v65[:, sc, :, 0:64],
                        in0=ps[:].rearrange("p (a b) -> p a b", a=8),
                        in1=bvb_sb[:].rearrange("p (a b) -> p a b", a=8),
                        op=OP.add,
                    )

            # ---------- attention (per head) ----------
            with tc.tile_pool(name="hp", bufs=2) as hp, tc.tile_pool(
                name="sps", bufs=2, space="PSUM"
            ) as sps, tc.tile_pool(name="cps", bufs=2, space="PSUM") as cps, tc.tile_pool(
                name="aps", bufs=1, space="PSUM"
            ) as aps, tc.tile_pool(name="dstage", bufs=2, space="DRAM") as dstage:
                for h in range(H):
                    hc, hr = h // 2, (h % 2) * 64

                    # transposed-orientation scores -> exp -> ET (k on partitions)
                    ET = hp.tile([128, 8, S], f16, tag="ET")
                    for kc in range(8):
                        ps = sps.tile([128, 1024], f32, tag="sc")
                        for qh in range(2):
                            nc.tensor.matmul(
                                ps[:, qh * 512 : (qh + 1) * 512],
                                kT[hr : hr + 64, hc, kc * 128 : (kc + 1) * 128],
                                qT[hr : hr + 64, hc, qh * 512 : (qh + 1) * 512],
                                start=True,
                                stop=True,
                            )
                        Et = hp.tile([128, S], f16, tag="Et")
                        nc.scalar.activation(out=Et, in_=ps, func=AF.Exp)
                        nc.vector.tensor_tensor(
                            out=ET[:, kc], in0=Et, in1=emadjT_sb[:, kc], op=OP.mult
                        )

                    # ctx^T = v^T @ E^T; ones column of v gives denominators in row 64
                    rcrow = hp.tile([65, 1024], f32, tag="rcrow")
                    pcs = []
                    for qh in range(2):
                        pc = cps.tile([65, 512], f32, tag="ctx")
                        for kc in range(8):
                            nc.tensor.matmul(
                                pc,
                                v65[:, kc, h, :],
                                ET[:, kc, qh * 512 : (qh + 1) * 512],
                                start=(kc == 0),
                                stop=(kc == 7),
                            )
                        nc.vector.reciprocal(
                            out=rcrow[64:65, qh * 512 : (qh + 1) * 512],
                            in_=pc[64:65, :],
                        )
                        pcs.append(pc)

                    # broadcast 1/denom along free dim for ctx normalize
                    rrow = dstage.tile([1, 1024], f32, tag="rrow")
                    nc.sync.dma_start(out=rrow[:], in_=rcrow[64:65, :])
                    rb = hp.tile([64, 1024], f32, tag="rb")
                    nc.gpsimd.dma_start(out=rb, in_=rrow[:].to_broadcast([64, 1024]))
                    for qh in range(2):
                        nc.vector.tensor_tensor(
                            out=ctxT_sb[hr : hr + 64, hc, qh * 512 : (qh + 1) * 512],
                            in0=pcs[qh][0:64, :],
                            in1=rb[:, qh * 512 : (qh + 1) * 512],
                            op=OP.mult,
                        )

                    # 1/denom in q-partition layout for attn normalize
                    rq8 = hp.tile([8, 128], f32, tag="rq8")
                    nc.sync.dma_start(
                        out=rq8, in_=rrow[:].rearrange("o (p f) -> (o p) f", p=8)
                    )
                    psT = aps.tile([128, 8], f32, tag="rsT")
                    nc.tensor.transpose(psT, rq8, ident_sb[0:8, 0:8])
                    rcp = hp.tile([128, 8], f32, tag="rcp")
                    nc.vector.tensor_copy(rcp[:], psT[:])

                    # natural-orientation scores -> attn output
                    for qc in range(8):
                        ps = sps.tile([128, 1024], f32, tag="sc")
                        for kh in range(2):
                            nc.tensor.matmul(
                                ps[:, kh * 512 : (kh + 1) * 512],
                                qT[hr : hr + 64, hc, qc * 128 : (qc + 1) * 128],
                                kT[hr : hr + 64, hc, kh * 512 : (kh + 1) * 512],
                                start=True,
                                stop=True,
                            )
                        Eq = hp.tile([128, S], f16, tag="E")
                        nc.scalar.activation(out=Eq, in_=ps, func=AF.Exp)
                        ast = hp.tile([128, S], f32, tag="ast")
                        nc.vector.scalar_tensor_tensor(
                            out=ast,
                            in0=Eq,
                            scalar=rcp[:, qc : qc + 1],
                            in1=emadj_sb[:, qc],
                            op0=OP.mult,
                            op1=OP.mult,
                        )
                        nc.sync.dma_start(
                            out=attn_d[h, qc * 128 : (qc + 1) * 128, :], in_=ast
                        )

            # ---------- output projection + residual + LayerNorm ----------
            with tc.tile_pool(name="op", bufs=2) as op, tc.tile_pool(
                name="ops", bufs=2, space="PSUM"
            ) as ops:
                for sc in range(8):
                    po = ops.tile([128, 512], f32, tag="o")
                    for c in range(4):
                        nc.tensor.matmul(
                            po,
                            ctxT_sb[:, c, sc * 128 : (sc + 1) * 128],
                            wo_sb[:, c],
                            start=(c == 0),
                            stop=(c == 3),
                        )
                    x = op.tile([128, 512], f32, tag="x")
                    nc.vector.tensor_tensor(
                        out=x, in0=po, in1=qres_sb[:, sc], op=OP.add
                    )
                    st = op.tile([128, 6], f32, tag="st")
                    nc.vector.bn_stats(out=st, in_=x)
                    ag = op.tile([128, 2], f32, tag="ag")
                    nc.vector.bn_aggr(out=ag, in_=st)
                    sd = op.tile([128, 1], f32, tag="sd")
                    nc.scalar.activation(
                        out=sd, in_=ag[:, 1:2], func=AF.Sqrt, bias=eps_sb[:]
                    )
                    rstd = op.tile([128, 1], f32, tag="rstd")
                    nc.vector.reciprocal(out=rstd, in_=sd)
                    y = op.tile([128, 512], f32, tag="y")
                    nc.vector.tensor_scalar(
                        out=y,
                        in0=x,
                        scalar1=ag[:, 0:1],
                        scalar2=rstd,
                        op0=OP.subtract,
                        op1=OP.mult,
                    )
                    y2 = op.tile([128, 512], f32, tag="y2")
                    nc.vector.tensor_tensor(out=y2, in0=y, in1=gb_sb, op=OP.mult)
                    o = op.tile([128, 512], f32, tag="oo")
                    nc.vector.tensor_tensor(out=o, in0=y2, in1=lnbb_sb, op=OP.add)
                    nc.sync.dma_start(out=out_d[sc * 128 : (sc + 1) * 128, :], in_=o)

    nc.finalize()
    return nc


def _get_program():
    if "nc" not in _CACHE:
        _CACHE["nc"] = _build()
    return _CACHE["nc"]


def make_in_maps(Q, K, V, attn_mask, adjoin_matrix, Wq, bq, Wk, bk, Wv, bv, Wo, bo, ln_g, ln_b):
    scale = np.float32(0.125)
    wq_s = (np.asarray(Wq, np.float32) * scale).astype(np.float16)
    wk_s = np.asarray(Wk, np.float32).astype(np.float16)
    wv_s = np.asarray(Wv, np.float32).astype(np.float16)
    wo_s = np.asarray(Wo, np.float32).astype(np.float16)
    bqc = np.ascontiguousarray(
        (np.asarray(bq, np.float32) * scale).reshape(4, 128).T
    )
    bkc = np.ascontiguousarray(np.asarray(bk, np.float32).reshape(4, 128).T)
    bvr = np.asarray(bv, np.float32).reshape(1, D)
    gr = np.asarray(ln_g, np.float32).reshape(1, D)
    lnbr = np.asarray(ln_b, np.float32).reshape(1, D)
    bo32 = np.asarray(bo, np.float32)

    madj_all = np.where(np.asarray(attn_mask, bool), NEG, np.float32(0.0)).astype(
        np.float32
    ) + np.asarray(adjoin_matrix, np.float32)[:, 0]
    emadj16 = np.exp(madj_all).astype(np.float16)

    in_maps = []
    for b in range(B):
        in_maps.append(
            {
                "qb": np.asarray(Q[b], np.float32).astype(np.float16),
                "kb": np.asarray(K[b], np.float32).astype(np.float16),
                "vb": np.asarray(V[b], np.float32).astype(np.float16),
                "qres": (np.asarray(Q[b], np.float32) + bo32[None, :]).astype(
                    np.float32
                ),
                "emadj": np.ascontiguousarray(emadj16[b]),
                "emadjT": np.ascontiguousarray(emadj16[b].T),
                "wq": wq_s,
                "wk": wk_s,
                "wv": wv_s,
                "wo": wo_s,
                "bqc": bqc,
                "bkc": bkc,
                "bvr": bvr,
                "gr": gr,
                "lnbr": lnbr,
            }
        )
    return in_maps


def kernel(Q, K, V, attn_mask, adjoin_matrix, Wq, bq, Wk, bk, Wv, bv, Wo, bo, ln_g, ln_b):
    from concourse import bass_utils

    nc = _get_program()
    in_maps = make_in_maps(
        Q, K, V, attn_mask, adjoin_matrix, Wq, bq, Wk, bk, Wv, bv, Wo, bo, ln_g, ln_b
    )
    res = bass_utils.run_bass_kernel_spmd(nc, in_maps, core_ids=list(range(B)))
    out = np.stack([r["out_o"] for r in res.results])
    attn = np.stack([r["attn_o"] for r in res.results])
    return out, attn


# revision 34
# speedup vs baseline: 1.4059x; 1.3749x over previous
import sys

if "/opt/trn_rl_repo" not in sys.path:
    sys.path.insert(0, "/opt/trn_rl_repo")

import numpy as np

B, S, D, H = 8, 1024, 512, 8
DK = 64
NEG = np.float32(-1e9)

_CACHE = {}


def _build():
    import concourse.bacc as bacc
    import concourse.mybir as mybir
    import concourse.tile as tile

    f16 = mybir.dt.float16
    f32 = mybir.dt.float32
    f32r = mybir.dt.float32r
    AF = mybir.ActivationFunctionType
    OP = mybir.AluOpType

    nc = bacc.Bacc("TRN2", target_bir_lowering=False, debug=False)

    qb_d = nc.dram_tensor("qb", (S, D), f16, kind="ExternalInput")
    kb_d = nc.dram_tensor("kb", (S, D), f16, kind="ExternalInput")
    vb_d = nc.dram_tensor("vb", (S, D), f16, kind="ExternalInput")
    qres_d = nc.dram_tensor("qres", (S, D), f32, kind="ExternalInput")
    emadjT_d = nc.dram_tensor("emadjT", (S, S), f16, kind="ExternalInput")
    wq_d = nc.dram_tensor("wq", (D, D), f16, kind="ExternalInput")
    wk_d = nc.dram_tensor("wk", (D, D), f16, kind="ExternalInput")
    wv_d = nc.dram_tensor("wv", (D, D), f16, kind="ExternalInput")
    wo_d = nc.dram_tensor("wo", (D, D), f16, kind="ExternalInput")
    bq_d = nc.dram_tensor("bqc", (128, 4), f32, kind="ExternalInput")
    bk_d = nc.dram_tensor("bkc", (128, 4), f32, kind="ExternalInput")
    bv_d = nc.dram_tensor("bvr", (1, D), f32, kind="ExternalInput")
    g_d = nc.dram_tensor("gr", (1, D), f32, kind="ExternalInput")
    lnb_d = nc.dram_tensor("lnbr", (1, D), f32, kind="ExternalInput")

    out_d = nc.dram_tensor("out_o", (S, D), f32, kind="ExternalOutput")
    # attn^T per head: [h, k, q] in f16; host transposes + upcasts
    attn_d = nc.dram_tensor("attn_o", (H, S, S), f16, kind="ExternalOutput")

    with tile.TileContext(nc) as tc:
        with tc.tile_pool(name="persist", bufs=1) as persist:
            qT = persist.tile([128, 4, S], f16)
            kT = persist.tile([128, 4, S], f16)
            v65 = persist.tile([128, 8, 8, 65], f16)
            emadjT_sb = persist.tile([128, 8, S], f16)
            qres_sb = persist.tile([128, 8, D], f32)
            ctxT_sb = persist.tile([128, 4, S], f16)
            wo_sb = persist.tile([128, 4, D], f16)
            gb_sb = persist.tile([128, D], f32)
            lnbb_sb = persist.tile([128, D], f32)
            bvb_sb = persist.tile([128, D], f32)
            bq_sb = persist.tile([128, 4], f32)
            bk_sb = persist.tile([128, 4], f32)
            eps_sb = persist.tile([128, 1], f32)
            nc.vector.memset(eps_sb[:], 1e-6)
            nc.vector.memset(v65[:], 1.0)

            nc.sync.dma_start(
                out=emadjT_sb, in_=emadjT_d[:].rearrange("(kc p) q -> p kc q", p=128)
            )
            nc.sync.dma_start(
                out=qres_sb, in_=qres_d[:].rearrange("(sc p) d -> p sc d", p=128)
            )
            nc.sync.dma_start(
                out=wo_sb, in_=wo_d[:].rearrange("(c p) n -> p c n", p=128)
            )
            nc.gpsimd.dma_start(out=gb_sb, in_=g_d[:].to_broadcast([128, D]))
            nc.gpsimd.dma_start(out=lnbb_sb, in_=lnb_d[:].to_broadcast([128, D]))
            nc.gpsimd.dma_start(out=bvb_sb, in_=bv_d[:].to_broadcast([128, D]))
            nc.sync.dma_start(out=bq_sb, in_=bq_d[:])
            nc.sync.dma_start(out=bk_sb, in_=bk_d[:])

            # ---------- projections ----------
            with tc.tile_pool(name="pin", bufs=1) as pin, tc.tile_pool(
                name="psp", bufs=2, space="PSUM"
            ) as psp:
                QT = pin.tile([128, 4, S], f16)
                KT = pin.tile([128, 4, S], f16)
                VT = pin.tile([128, 4, S], f16)
                wq_sb = pin.tile([128, 4, D], f16)
                wk_sb = pin.tile([128, 4, D], f16)
                wv_sb = pin.tile([128, 4, D], f16)
                nc.sync.dma_start(
                    out=wq_sb, in_=wq_d[:].rearrange("(c p) n -> p c n", p=128)
                )
                nc.sync.dma_start(
                    out=wk_sb, in_=wk_d[:].rearrange("(c p) n -> p c n", p=128)
                )
                nc.sync.dma_start(
                    out=wv_sb, in_=wv_d[:].rearrange("(c p) n -> p c n", p=128)
                )
                for dc in range(4):
                    nc.sync.dma_start_transpose(
                        QT[:, dc], qb_d[:, dc * 128 : (dc + 1) * 128]
                    )
                    nc.sync.dma_start_transpose(
                        KT[:, dc], kb_d[:, dc * 128 : (dc + 1) * 128]
                    )
                    nc.sync.dma_start_transpose(
                        VT[:, dc], vb_d[:, dc * 128 : (dc + 1) * 128]
                    )

                for src, wsb, bsb, dst in (
                    (QT, wq_sb, bq_sb, qT),
                    (KT, wk_sb, bk_sb, kT),
                ):
                    for hc in range(4):
                        for sh in range(2):
                            ps = psp.tile([128, 512], f32, tag="proj")
                            for dc in range(4):
                                nc.tensor.matmul(
                                    ps,
                                    wsb[:, dc, hc * 128 : (hc + 1) * 128],
                                    src[:, dc, sh * 512 : (sh + 1) * 512],
                                    start=(dc == 0),
                                    stop=(dc == 3),
                                )
                            nc.vector.tensor_scalar(
                                out=dst[:, hc, sh * 512 : (sh + 1) * 512],
                                in0=ps,
                                scalar1=bsb[:, hc : hc + 1],
                                scalar2=None,
                                op0=OP.add,
                            )
                for sc in range(8):
                    ps = psp.tile([128, 512], f32, tag="proj")
                    for dc in range(4):
                        nc.tensor.matmul(
                            ps,
                            VT[:, dc, sc * 128 : (sc + 1) * 128],
                            wv_sb[:, dc],
                            start=(dc == 0),
                            stop=(dc == 3),
                        )
                    nc.vector.tensor_tensor(
                        out=v65[:, sc, :, 0:64],
                        in0=ps[:].rearrange("p (a b) -> p a b", a=8),
                        in1=bvb_sb[:].rearrange("p (a b) -> p a b", a=8),
                        op=OP.add,
                    )

            # ---------- attention (per head) ----------
            with tc.tile_pool(name="hp", bufs=2) as hp, tc.tile_pool(
                name="sps", bufs=3, space="PSUM"
            ) as sps, tc.tile_pool(name="cps", bufs=2, space="PSUM") as cps, tc.tile_pool(
                name="dstage", bufs=2, space="DRAM"
            ) as dstage:
                for h in range(H):
                    hc, hr = h // 2, (h % 2) * 64

                    # transposed-orientation scores -> exp -> ET (k on partitions)
                    ET = hp.tile([128, 8, S], f16, tag="ET")
                    for kc in range(8):
                        ps = sps.tile([128, 1024], f32, tag="sc")
                        for qh in range(2):
                            nc.tensor.matmul(
                                ps[:, qh * 512 : (qh + 1) * 512],
                                kT[hr : hr + 64, hc, kc * 128 : (kc + 1) * 128],
                                qT[hr : hr + 64, hc, qh * 512 : (qh + 1) * 512],
                                start=True,
                                stop=True,
                            )
                        Et = hp.tile([128, S], f16, tag="Et")
                        nc.scalar.activation(out=Et, in_=ps, func=AF.Exp)
                        nc.vector.tensor_tensor(
                            out=ET[:, kc], in0=Et, in1=emadjT_sb[:, kc], op=OP.mult
                        )

                    # ctx^T = v^T @ E^T; ones column of v gives denominators in row 64
                    sbden = hp.tile([65, 1024], f32, tag="sbden")
                    pcs = []
                    for qh in range(2):
                        pc = cps.tile([65, 512], f32, tag="ctx")
                        for kc in range(8):
                            nc.tensor.matmul(
                                pc,
                                v65[:, kc, h, :],
                                ET[:, kc, qh * 512 : (qh + 1) * 512],
                                start=(kc == 0),
                                stop=(kc == 7),
                            )
                        nc.scalar.copy(
                            out=sbden[64:65, qh * 512 : (qh + 1) * 512],
                            in_=pc[64:65, :],
                        )
                        pcs.append(pc)

                    # broadcast denominators to all partitions, one reciprocal
                    rrow = dstage.tile([1, 1024], f32, tag="rrow")
                    nc.sync.dma_start(out=rrow[:], in_=sbden[64:65, :])
                    den128 = hp.tile([128, 1024], f32, tag="den")
                    nc.gpsimd.dma_start(
                        out=den128, in_=rrow[:].to_broadcast([128, 1024])
                    )
                    rb = hp.tile([128, 1024], f16, tag="rb")
                    with nc.allow_low_precision(reason="f16 attn output"):
                        nc.vector.reciprocal(out=rb, in_=den128)
                        for qh in range(2):
                            nc.vector.tensor_tensor(
                                out=ctxT_sb[
                                    hr : hr + 64, hc, qh * 512 : (qh + 1) * 512
                                ],
                                in0=pcs[qh][0:64, :],
                                in1=rb[0:64, qh * 512 : (qh + 1) * 512],
                                op=OP.mult,
                            )

                        # normalized attn^T rows -> DRAM (f16, [h, k, q])
                        for kc in range(8):
                            an = hp.tile([128, S], f16, tag="AN")
                            nc.vector.tensor_tensor(
                                out=an, in0=ET[:, kc], in1=rb, op=OP.mult
                            )
                            nc.sync.dma_start(
                                out=attn_d[h, kc * 128 : (kc + 1) * 128, :], in_=an
                            )

            # ---------- output projection + residual + LayerNorm ----------
            with tc.tile_pool(name="op", bufs=2) as op, tc.tile_pool(
                name="ops", bufs=2, space="PSUM"
            ) as ops:
                for sc in range(8):
                    po = ops.tile([128, 512], f32, tag="o")
                    for c in range(4):
                        nc.tensor.matmul(
                            po,
                            ctxT_sb[:, c, sc * 128 : (sc + 1) * 128],
                            wo_sb[:, c],
                            start=(c == 0),
                            stop=(c == 3),
                        )
                    x = op.tile([128, 512], f32, tag="x")
                    nc.vector.tensor_tensor(
                        out=x, in0=po, in1=qres_sb[:, sc], op=OP.add
                    )
                    st = op.tile([128, 6], f32, tag="st")
                    nc.vector.bn_stats(out=st, in_=x)
                    ag = op.tile([128, 2], f32, tag="ag")
                    nc.vector.bn_aggr(out=ag, in_=st)
                    sd = op.tile([128, 1], f32, tag="sd")
                    nc.scalar.activation(
                        out=sd, in_=ag[:, 1:2], func=AF.Sqrt, bias=eps_sb[:]
                    )
                    rstd = op.tile([128, 1], f32, tag="rstd")
                    nc.vector.reciprocal(out=rstd, in_=sd)
                    y = op.tile([128, 512], f32, tag="y")
                    nc.vector.tensor_scalar(
                        out=y,
                        in0=x,
                        scalar1=ag[:, 0:1],
                        scalar2=rstd,
                        op0=OP.subtract,
                        op1=OP.mult,
                    )
                    y2 = op.tile([128, 512], f32, tag="y2")
                    nc.vector.tensor_tensor(out=y2, in0=y, in1=gb_sb, op=OP.mult)
                    o = op.tile([128, 512], f32, tag="oo")
                    nc.vector.tensor_tensor(out=o, in0=y2, in1=lnbb_sb, op=OP.add)
                    nc.sync.dma_start(out=out_d[sc * 128 : (sc + 1) * 128, :], in_=o)

    nc.finalize()
    return nc


def _get_program():
    if "nc" not in _CACHE:
        _CACHE["nc"] = _build()
    return _CACHE["nc"]


def make_in_maps(Q, K, V, attn_mask, adjoin_matrix, Wq, bq, Wk, bk, Wv, bv, Wo, bo, ln_g, ln_b):
    scale = np.float32(0.125)
    wq_s = (np.asarray(Wq, np.float32) * scale).astype(np.float16)
    wk_s = np.asarray(Wk, np.float32).astype(np.float16)
    wv_s = np.asarray(Wv, np.float32).astype(np.float16)
    wo_s = np.asarray(Wo, np.float32).astype(np.float16)
    bqc = np.ascontiguousarray(
        (np.asarray(bq, np.float32) * scale).reshape(4, 128).T
    )
    bkc = np.ascontiguousarray(np.asarray(bk, np.float32).reshape(4, 128).T)
    bvr = np.asarray(bv, np.float32).reshape(1, D)
    gr = np.asarray(ln_g, np.float32).reshape(1, D)
    lnbr = np.asarray(ln_b, np.float32).reshape(1, D)
    bo32 = np.asarray(bo, np.float32)

    madj_all = np.where(np.asarray(attn_mask, bool), NEG, np.float32(0.0)).astype(
        np.float32
    ) + np.asarray(adjoin_matrix, np.float32)[:, 0]
    emadj16 = np.exp(madj_all).astype(np.float16)

    in_maps = []
    for b in range(B):
        in_maps.append(
            {
                "qb": np.asarray(Q[b], np.float32).astype(np.float16),
                "kb": np.asarray(K[b], np.float32).astype(np.float16),
                "vb": np.asarray(V[b], np.float32).astype(np.float16),
                "qres": (np.asarray(Q[b], np.float32) + bo32[None, :]).astype(
                    np.float32
                ),
                "emadjT": np.ascontiguousarray(emadj16[b].T),
                "wq": wq_s,
                "wk": wk_s,
                "wv": wv_s,
                "wo": wo_s,
                "bqc": bqc,
                "bkc": bkc,
                "bvr": bvr,
                "gr": gr,
                "lnbr": lnbr,
            }
        )
    return in_maps


def kernel(Q, K, V, attn_mask, adjoin_matrix, Wq, bq, Wk, bk, Wv, bv, Wo, bo, ln_g, ln_b):
    from concourse import bass_utils

    nc = _get_program()
    in_maps = make_in_maps(
        Q, K, V, attn_mask, adjoin_matrix, Wq, bq, Wk, bk, Wv, bv, Wo, bo, ln_g, ln_b
    )
    res = bass_utils.run_bass_kernel_spmd(nc, in_maps, core_ids=list(range(B)))
    out = np.stack([r["out_o"] for r in res.results])
    attn = np.stack(
        [np.swapaxes(r["attn_o"], 1, 2) for r in res.results]
    ).astype(np.float32)
    return out, attn
